# revision 1
# baseline (speedup 1.0000x reference)
"""Trainium2 Bass kernel for nn_AttentionContextLayer (Bahdanau additive attention).

Per batch b:
  qp = X @ Wp + bp          [512,128]
  qh = qp @ Wq + bq         [512,128]
  vh = V @ Wv + bv          [256,128]
  score[q,t] = sum_u v[u]*tanh(qh[q,u]+vh[t,u])   (+vb, which cancels in softmax)
  attn = softmax_t(score + (mask-1)*1e9)
  ctx  = attn @ V
  out  = concat([X, ctx], -1)                      [512,512]

Sharding: data-parallel over B=8, one batch per NeuronCore. Per core:
  - transposed layouts throughout: qhT [u,q] fp32, vhT [u,t] fp32
  - per t-block of 16: DVE tensor_scalar_add builds S_t = qhT + vhT[:,t]
    (15 t's; the 16th goes through ScalarE's fused bias path), ScalarE tanh
    over [128, 15*512] fp32 blocks -> bf16 (the ~110us/core compute roofline),
    PE contracts with v via a shifted one-hot-scaled bf16 stationary matrix that
    accumulates score row t onto PSUM partition t -> scoreT [t,q] tiles.
  - exp with mask folded in as a per-partition bias of (mask-1)*1e9 -> bf16,
  - bf16 context matmul with ones-augmented values -> softmax denominator free;
    the first half (t 0..127) is issued mid-loop to shorten the tail,
  - DVE reciprocal + per-partition scale, DMA out.
"""

import numpy as np

import concourse.bass as bass
import concourse.mybir as mybir
import concourse.tile as tile
from concourse import bacc
from concourse.bass import ds, ts
from concourse.bass_utils import run_bass_kernel_spmd
from concourse.masks import make_identity

TQ, DQ = 512, 256
TV, DV = 256, 256
U = 128
TB = 16         # t-block size for tanh batching
F32 = mybir.dt.float32
BF16 = mybir.dt.bfloat16
AF = mybir.ActivationFunctionType


def build_graph():
    nc = bacc.Bacc(None)

    x_ext = nc.declare_dram_parameter("x", [TQ, DQ], F32, isOutput=False)
    vals_ext = nc.declare_dram_parameter("vals", [TV, DV], F32, isOutput=False)
    # wcat: [wp0 | wp1 | wq | wv0 | wv1], each [128,128]
    wcat_ext = nc.declare_dram_parameter("wcat", [U, 5 * U], F32, isOutput=False)
    # ccat: [vshift(256) | bp | bq | bv | embias0 | embias1]
    ccat_ext = nc.declare_dram_parameter("ccat", [U, 261], F32, isOutput=False)
    out_ext = nc.declare_dram_parameter("out", [TQ, DQ + DV], F32, isOutput=True)

    NQT = TQ // 128   # 4 q tiles
    NTT = TV // 128   # 2 t tiles
    NDT = DQ // 128   # 2 d tiles

    with tile.TileContext(nc) as tc:
        with (
            tc.tile_pool(name="const", bufs=1) as cp,
            tc.tile_pool(name="spool", bufs=2) as s_pool,
            tc.tile_pool(name="tpool", bufs=2) as t_pool,
            tc.tile_pool(name="ps0", bufs=2, space="PSUM") as ps0,
            tc.tile_pool(name="score_ps", bufs=1, space="PSUM") as score_ps,
            tc.tile_pool(name="ctx_ps", bufs=1, space="PSUM") as ctx_ps,
            tc.tile_pool(name="small", bufs=4) as small_pool,
            tc.tile_pool(name="ctx_sb", bufs=2) as ctx_pool,
        ):
            # ---------------- stage 0: loads (few, batched DMAs) ----------
            wcat_sb = cp.tile([128, 5 * U], F32, tag="wcat")
            nc.sync.dma_start(out=wcat_sb, in_=wcat_ext[:, :])
            ccat_sb = cp.tile([128, 261], F32, tag="ccat")
            nc.sync.dma_start(out=ccat_sb, in_=ccat_ext[:, :])

            x_sb = []
            for qt in range(NQT):
                t_ = cp.tile([128, DQ], F32, tag=f"x{qt}")
                nc.sync.dma_start(out=t_, in_=x_ext[qt * 128:(qt + 1) * 128, :])
                x_sb.append(t_)
            vals_sb = []
            for tt in range(NTT):
                t_ = cp.tile([128, DV + 1], F32, tag=f"vals{tt}")
                nc.sync.dma_start(
                    out=t_[:, 0:DV], in_=vals_ext[tt * 128:(tt + 1) * 128, :])
                nc.vector.memset(t_[:, ds(DV, 1)], 1.0)  # ones col -> denom
                vals_sb.append(t_)

            # first half of output is just X: direct HBM->HBM, off the sync queue
            nc.gpsimd.dma_start(out=out_ext[:, 0:DQ], in_=x_ext[:, :])

            identity = cp.tile([128, 128], F32, tag="identity")
            make_identity(nc, identity)

            wcat_bf = cp.tile([128, 5 * U], BF16, tag="wcat_bf")
            nc.vector.tensor_copy(wcat_bf, wcat_sb)
            wp_bf = [wcat_bf[:, ts(dt, U)] for dt in range(NDT)]
            wq_bf = wcat_bf[:, ds(2 * U, U)]
            wv_bf = [wcat_bf[:, ds((3 + dt) * U, U)] for dt in range(NDT)]

            vshift_sb = cp.tile([128, 256], BF16, tag="vshift")
            nc.vector.tensor_copy(vshift_sb, ccat_sb[:, ds(0, 256)])
            bp_ap = ccat_sb[:, ds(256, 1)]
            bq_ap = ccat_sb[:, ds(257, 1)]
            bv_ap = ccat_sb[:, ds(258, 1)]
            embias_ap = [ccat_sb[:, ds(259 + tt, 1)] for tt in range(NTT)]

            vals_bf = []
            for tt in range(NTT):
                t_ = cp.tile([128, DV + 1], BF16, tag=f"vals_bf{tt}")
                nc.vector.tensor_copy(t_, vals_sb[tt])
                vals_bf.append(t_)

            # ---------------- stage 0: transposes (PE) --------------------
            xt_sb = []
            for dt in range(NDT):
                ps = ps0.tile([128, TQ], F32, tag="ps0")
                for qt in range(NQT):
                    nc.tensor.transpose(
                        ps[:, ts(qt, 128)], x_sb[qt][:, ts(dt, 128)], identity)
                t_ = cp.tile([128, TQ], BF16, tag=f"xt{dt}")
                nc.vector.tensor_copy(t_, ps)
                xt_sb.append(t_)

            valsT_sb = []
            for dt in range(NDT):
                ps = ps0.tile([128, TV], F32, tag="ps0")
                for tt in range(NTT):
                    nc.tensor.transpose(
                        ps[:, ts(tt, 128)], vals_sb[tt][:, ts(dt, 128)], identity)
                t_ = cp.tile([128, TV], BF16, tag=f"valsT{dt}")
                nc.vector.tensor_copy(t_, ps)
                valsT_sb.append(t_)

            # ---------------- stage 0: projections (bf16, transposed) -----
            ps_qp = ps0.tile([128, TQ], F32, tag="ps0")
            for dt in range(NDT):
                nc.tensor.matmul(ps_qp, wp_bf[dt], xt_sb[dt],
                                 start=(dt == 0), stop=(dt == NDT - 1))
            qp_sb = cp.tile([128, TQ], BF16, tag="qp")
            nc.vector.tensor_scalar_add(out=qp_sb, in0=ps_qp, scalar1=bp_ap)

            ps_qh = ps0.tile([128, TQ], F32, tag="ps0")
            nc.tensor.matmul(ps_qh, wq_bf, qp_sb, start=True, stop=True)
            qh_sb = cp.tile([128, TQ], F32, tag="qh")
            nc.vector.tensor_scalar_add(out=qh_sb, in0=ps_qh, scalar1=bq_ap)

            ps_vh = ps0.tile([128, TV], F32, tag="ps0")
            for dt in range(NDT):
                nc.tensor.matmul(ps_vh, wv_bf[dt], valsT_sb[dt],
                                 start=(dt == 0), stop=(dt == NDT - 1))
            vh_sb = cp.tile([128, TV], F32, tag="vh")
            nc.vector.tensor_scalar_add(out=vh_sb, in0=ps_vh, scalar1=bv_ap)

            # ---------------- stage 1: tanh + v-contraction ---------------
            score_psum = [score_ps.tile([128, TQ], F32, tag=f"score{tt}", name=f"score{tt}")
                          for tt in range(NTT)]
            numer_sb = [cp.tile([128, TQ], BF16, tag=f"numer{tt}", name=f"numer{tt}")
                        for tt in range(NTT)]
            ctx_psum = [ctx_ps.tile([128, DV + 1], F32, tag=f"ctx{qt}", name=f"ctx{qt}")
                        for qt in range(NQT)]

            nblk = TV // TB
            for blk in range(nblk):
                s_t = s_pool.tile([128, (TB - 1) * TQ], F32, tag="s")
                for j in range(TB - 1):
                    t = blk * TB + j
                    nc.vector.tensor_scalar_add(
                        out=s_t[:, ts(j, TQ)], in0=qh_sb,
                        scalar1=vh_sb[:, ds(t, 1)])
                th_t = t_pool.tile([128, TB * TQ], BF16, tag="t")
                # bulk tanh for the 15 DVE-built columns...
                nc.scalar.activation(th_t[:, ds(0, (TB - 1) * TQ)], s_t, AF.Tanh)
                # ...and the 16th via ScalarE's fused  tanh(qh + vh[:,t])
                t_last = blk * TB + TB - 1
                nc.scalar.activation(
                    th_t[:, ds((TB - 1) * TQ, TQ)], qh_sb, AF.Tanh,
                    bias=vh_sb[:, ds(t_last, 1)])
                for j in range(TB):
                    t = blk * TB + j
                    tt, tl = t // 128, t % 128
                    nc.tensor.matmul(
                        score_psum[tt],
                        vshift_sb[:, ds(128 - tl, 128)],
                        th_t[:, ts(j, TQ)],
                        start=(tl == 0), stop=(tl == 127))
                if t_last % 128 == 127:
                    tt = t_last // 128
                    nc.scalar.activation(
                        numer_sb[tt], score_psum[tt], AF.Exp,
                        bias=embias_ap[tt])
                    # issue this half's context matmuls now (hides them
                    # behind the remaining tanh blocks for tt=0)
                    for qt in range(NQT):
                        nc.tensor.matmul(
                            ctx_psum[qt], numer_sb[tt][:, ts(qt, 128)],
                            vals_bf[tt],
                            start=(tt == 0), stop=(tt == NTT - 1))

            # ---------------- stage 2: normalize + store ------------------
            for qt in range(NQT):
                recip = small_pool.tile([128, 1], F32, tag="recip")
                nc.vector.reciprocal(recip, ctx_psum[qt][:, ds(DV, 1)])
                ctx_sb = ctx_pool.tile([128, DV], F32, tag="ctx_sb")
                nc.vector.tensor_scalar_mul(
                    out=ctx_sb, in0=ctx_psum[qt][:, ds(0, DV)], scalar1=recip)
                nc.sync.dma_start(
                    out=out_ext[qt * 128:(qt + 1) * 128, DQ:DQ + DV],
                    in_=ctx_sb)

    nc.compile()
    return nc


def _make_in_maps(inputs):
    query_seq = np.asarray(inputs["query_seq"], np.float32)
    values = np.asarray(inputs["values"], np.float32)
    mask = np.asarray(inputs["mask"])
    Wp = np.asarray(inputs["Wp"], np.float32)
    Wq = np.asarray(inputs["Wq"], np.float32)
    Wv = np.asarray(inputs["Wv"], np.float32)
    bp = np.asarray(inputs["bp"], np.float32).reshape(U, 1)
    bq = np.asarray(inputs["bq"], np.float32).reshape(U, 1)
    bv = np.asarray(inputs["bv"], np.float32).reshape(U, 1)
    v = np.asarray(inputs["v"], np.float32).reshape(U)
    # vb is a constant shift on all scores -> cancels in softmax; unused.

    wcat = np.ascontiguousarray(np.hstack(
        [Wp[0:128], Wp[128:256], Wq, Wv[0:128], Wv[128:256]]))
    vshift = np.zeros((U, 256), np.float32)
    vshift[:, 128] = v
    embias = (mask.astype(np.float32) - 1.0) * 1e9  # [8, 256]

    in_maps = []
    for i in range(8):
        ccat = np.ascontiguousarray(np.hstack(
            [vshift, bp, bq, bv,
             embias[i, 0:128].reshape(U, 1), embias[i, 128:256].reshape(U, 1)]
        ).astype(np.float32))
        in_maps.append({
            "x": np.ascontiguousarray(query_seq[i]),
            "vals": np.ascontiguousarray(values[i]),
            "wcat": wcat,
            "ccat": ccat,
        })
    return in_maps


def kernel(query_seq, values, mask, Wp, bp, Wq, bq, Wv, bv, v, vb):
    in_maps = _make_in_maps(dict(
        query_seq=query_seq, values=values, mask=mask, Wp=Wp, bp=bp,
        Wq=Wq, bq=bq, Wv=Wv, bv=bv, v=v, vb=vb))
    nc = build_graph()
    res = run_bass_kernel_spmd(nc, in_maps, core_ids=list(range(8)))
    out = np.stack([np.asarray(res.results[i]["out"]) for i in range(8)])
    return out.astype(np.float32)

